# revision 2
# baseline (speedup 1.0000x reference)
"""Trainium2 Bass kernel for nn_Correlation.

Computes avg off-diagonal |correlation| of per-sample channel correlation
matrices for x [32, 1024, 32, 32] fp32.

Strategy (data-parallel over N across 8 cores, 4 samples each):
  Per sample [C=1024, S=1024]:
    1. SWDGE cast-load DRAM fp32 -> SBUF bf16, natural [c, s] layout.
    2. bn_stats/bn_aggr per channel row -> mean, var (fp32);
       nrm = sqrt(S*var); sc = 1/(nrm + 1e-8).
    3. xn = (x - mean) * sc  (DVE tensor_scalar, bf16 out)  -- unit-norm rows.
    4. DMA-xbar transpose xn -> xT [s-group layout s = 8p + mid].
    5. Gram via PE: corr block = sum_k xT[k, m-cols]^T @ xT[k, n-cols], fp32 PSUM.
       Only upper-triangular blocks computed (symmetry), off-diag weighted 2x.
    6. ACT Abs + accum_out reduces each PSUM chunk into per-partition sums.
  Final: cross-partition reduce via ones-matmul -> scalar per core.
  Host: sum 8 core scalars, subtract diag contribution (32*1024), scale.
"""

import sys

import numpy as np

for _p in ("/opt/trn_rl_repo", "/root/.axon_site/_ro/trn_rl_repo"):
    if _p not in sys.path:
        sys.path.append(_p)

N_CORES = 8
NSAMP = 4  # samples per core
C = 1024
S = 1024
P = 128
CB = C // P  # channel blocks
KB = S // P  # contraction blocks (s-groups)
CHUNK = 512

_cache = {}


def _build_program():
    from contextlib import ExitStack

    import concourse.bass as bass
    import concourse.tile as tile
    from concourse import bacc, mybir

    dt = mybir.dt
    nc = bacc.Bacc("TRN2", target_bir_lowering=False, debug=False,
                   num_devices=N_CORES)

    x_dram = nc.dram_tensor("x", [NSAMP * C, S], dt.float32,
                            kind="ExternalInput")
    out_dram = nc.dram_tensor("out", [1, 1], dt.float32, kind="ExternalOutput")

    # Y accumulator columns: 0..7 diag-block sums (weight 1),
    # 8..18 off-diag chunk sums (weight 2)
    N_DIAG_COLS = 8
    N_OFF_COLS = 11
    NYC = N_DIAG_COLS + N_OFF_COLS

    with tile.TileContext(nc) as tc, ExitStack() as ctx:
        xbf_pool = ctx.enter_context(tc.tile_pool(name="xbf", bufs=2))
        st_pool = ctx.enter_context(tc.tile_pool(name="stats", bufs=2))
        xn_pool = ctx.enter_context(tc.tile_pool(name="xn", bufs=2))
        xt_pool = ctx.enter_context(tc.tile_pool(name="xt", bufs=2))
        ps_pool = ctx.enter_context(
            tc.tile_pool(name="psum", bufs=4, space="PSUM"))
        scr_pool = ctx.enter_context(tc.tile_pool(name="scr", bufs=3))
        fin_pool = ctx.enter_context(tc.tile_pool(name="fin", bufs=1))
        ps1_pool = ctx.enter_context(
            tc.tile_pool(name="psum1", bufs=1, space="PSUM"))

        Y = fin_pool.tile([P, NSAMP, NYC], dt.float32)
        ones = fin_pool.tile([P, 1], dt.float32)
        nc.vector.memset(ones, 1.0)

        for n in range(NSAMP):
            # ---- load (cast fp32 -> bf16) ----
            xbf = xbf_pool.tile([P, CB, S], dt.bfloat16)
            for cb in range(CB):
                r0 = n * C + cb * P
                nc.gpsimd.dma_start(out=xbf[:, cb, :],
                                    in_=x_dram[r0:r0 + P, :])

            # ---- per-channel stats ----
            st6 = st_pool.tile([P, CB, 2, 6], dt.float32)
            mv = st_pool.tile([P, CB, 2], dt.float32)
            nrm = st_pool.tile([P, CB], dt.float32)
            sc = st_pool.tile([P, CB], dt.float32)
            for cb in range(CB):
                nc.vector.bn_stats(out=st6[:, cb, 0, :], in_=xbf[:, cb, 0:512])
                nc.vector.bn_stats(out=st6[:, cb, 1, :], in_=xbf[:, cb, 512:S])
                nc.vector.bn_aggr(out=mv[:, cb, :], in_=st6[:, cb, :, :])
                # nrm = sqrt(S * var)
                nc.scalar.activation(out=nrm[:, cb:cb + 1],
                                     in_=mv[:, cb, 1:2],
                                     func=mybir.ActivationFunctionType.Sqrt,
                                     scale=float(S))
                nc.vector.tensor_scalar_add(out=nrm[:, cb:cb + 1],
                                            in0=nrm[:, cb:cb + 1],
                                            scalar1=1e-8)
                nc.vector.reciprocal(out=sc[:, cb:cb + 1],
                                     in_=nrm[:, cb:cb + 1])

            # ---- normalize: xn = (x - mean) * sc  (bf16 out) ----
            xn = xn_pool.tile([P, CB, S], dt.bfloat16)
            for cb in range(CB):
                nc.vector.tensor_scalar(out=xn[:, cb, :], in0=xbf[:, cb, :],
                                        scalar1=mv[:, cb, 0:1],
                                        scalar2=sc[:, cb:cb + 1],
                                        op0=mybir.AluOpType.subtract,
                                        op1=mybir.AluOpType.mult)

            # ---- transpose to [s-groups] layout: xT[p, mid, c], s = 8p+mid ----
            xT = xt_pool.tile([P, KB, C], dt.bfloat16)
            for cb in range(CB):
                nc.sync.dma_start(out=xT[:, :, cb * P:(cb + 1) * P],
                                  in_=xn[:, cb, :], transpose=True)

            # ---- Gram (upper-tri chunks) + Abs-accumulate ----
            off_col = N_DIAG_COLS
            for m in range(CB):
                d0 = m * P
                cs = d0
                ci = 0
                while cs < C:
                    w = min(CHUNK, C - cs)
                    ps = ps_pool.tile([P, CHUNK], dt.float32)
                    for kb in range(KB):
                        nc.tensor.matmul(ps[:, :w],
                                         xT[:, kb, d0:d0 + P],
                                         xT[:, kb, cs:cs + w],
                                         start=(kb == 0), stop=(kb == KB - 1))
                    scr = scr_pool.tile([P, CHUNK], dt.float32)
                    if ci == 0:
                        # chunk starts with the diagonal block
                        nc.scalar.activation(
                            out=scr[:, 0:P], in_=ps[:, 0:P],
                            func=mybir.ActivationFunctionType.Abs,
                            accum_out=Y[:, n, m:m + 1])
                        if w > P:
                            nc.scalar.activation(
                                out=scr[:, P:w], in_=ps[:, P:w],
                                func=mybir.ActivationFunctionType.Abs,
                                accum_out=Y[:, n, off_col:off_col + 1])
                            off_col += 1
                    else:
                        nc.scalar.activation(
                            out=scr[:, 0:w], in_=ps[:, 0:w],
                            func=mybir.ActivationFunctionType.Abs,
                            accum_out=Y[:, n, off_col:off_col + 1])
                        off_col += 1
                    cs += w
                    ci += 1
            assert off_col == NYC

        # ---- final reduction: total = sum_p (yd + 2*yo) ----
        yd = fin_pool.tile([P, 1], dt.float32)
        yo = fin_pool.tile([P, 1], dt.float32)
        r = fin_pool.tile([P, 1], dt.float32)
        nc.vector.reduce_sum(out=yd, in_=Y[:, :, 0:N_DIAG_COLS],
                             axis=mybir.AxisListType.XY)
        nc.vector.reduce_sum(out=yo, in_=Y[:, :, N_DIAG_COLS:NYC],
                             axis=mybir.AxisListType.XY)
        nc.vector.tensor_scalar(out=r, in0=yo, scalar1=2.0, scalar2=None,
                                op0=mybir.AluOpType.mult)
        nc.vector.tensor_add(out=r, in0=r, in1=yd)

        ps1 = ps1_pool.tile([1, 1], dt.float32)
        nc.tensor.matmul(ps1, r, ones, start=True, stop=True)
        res_sb = fin_pool.tile([1, 1], dt.float32)
        nc.vector.tensor_copy(out=res_sb, in_=ps1)
        nc.sync.dma_start(out=out_dram[:, :], in_=res_sb)

    nc.compile()
    return nc


def _get_program():
    if "nc" not in _cache:
        _cache["nc"] = _build_program()
    return _cache["nc"]


def kernel(**inputs) -> np.ndarray:
    from concourse.bass_utils import run_bass_kernel_spmd

    x = np.asarray(inputs["x"], dtype=np.float32).reshape(32, C, S)

    nc = _get_program()
    in_maps = [
        {"x": np.ascontiguousarray(
            x[i * NSAMP:(i + 1) * NSAMP].reshape(NSAMP * C, S))}
        for i in range(N_CORES)
    ]
    res = run_bass_kernel_spmd(nc, in_maps, core_ids=list(range(N_CORES)))
    total = sum(float(res.results[i]["out"][0, 0]) for i in range(N_CORES))
    total -= 32.0 * C  # remove diagonal (corr_cc ~= 1.0 each)
    num_combinations = C * (C - 1) // 2
    avg = total / num_combinations / 2.0 / 32.0
    return np.array(avg, dtype=np.float32)


# revision 3
# speedup vs baseline: 1.3767x; 1.3767x over previous
"""Trainium2 Bass kernel for nn_Correlation.

Computes avg off-diagonal |correlation| of per-sample channel correlation
matrices for x [32, 1024, 32, 32] fp32.

Strategy (data-parallel over N across 8 cores, 4 samples each):
  Per sample [C=1024, S=1024]:
    1. SWDGE cast-load DRAM fp32 -> SBUF bf16, natural [c, s] layout.
    2. bn_stats/bn_aggr per channel row -> mean, var (fp32);
       nrm = sqrt(S*var); sc = 1/(nrm + 1e-8).
    3. xn = (x - mean) * sc  (DVE tensor_scalar, bf16 out)  -- unit-norm rows.
    4. DMA-xbar transpose xn -> xT [s-group layout s = 8p + mid].
    5. Gram via PE: corr block = sum_k xT[k, m-cols]^T @ xT[k, n-cols], fp32 PSUM.
       Only upper-triangular blocks computed (symmetry), off-diag weighted 2x.
    6. ACT Abs + accum_out reduces each PSUM chunk into per-partition sums.
  Final: cross-partition reduce via ones-matmul -> scalar per core.
  Host: sum 8 core scalars, subtract diag contribution (32*1024), scale.
"""

import sys

import numpy as np

for _p in ("/opt/trn_rl_repo", "/root/.axon_site/_ro/trn_rl_repo"):
    if _p not in sys.path:
        sys.path.append(_p)

N_CORES = 8
NSAMP = 4  # samples per core
C = 1024
S = 1024
P = 128
CB = C // P  # channel blocks
KB = S // P  # contraction blocks (s-groups)
CHUNK = 512

_cache = {}


def _build_program():
    from contextlib import ExitStack

    import concourse.bass as bass
    import concourse.tile as tile
    from concourse import bacc, mybir

    dt = mybir.dt
    nc = bacc.Bacc("TRN2", target_bir_lowering=False, debug=False,
                   num_devices=N_CORES)

    x_dram = nc.dram_tensor("x", [NSAMP * C, S], dt.float32,
                            kind="ExternalInput")
    out_dram = nc.dram_tensor("out", [1, 1], dt.float32, kind="ExternalOutput")

    # Y accumulator columns: 0..7 diag-block sums (weight 1),
    # 8..18 off-diag chunk sums (weight 2)
    N_DIAG_COLS = 8
    N_OFF_COLS = 11
    NYC = N_DIAG_COLS + N_OFF_COLS

    with tile.TileContext(nc) as tc, ExitStack() as ctx:
        xbf_pool = ctx.enter_context(tc.tile_pool(name="xbf", bufs=3))
        st_pool = ctx.enter_context(tc.tile_pool(name="stats", bufs=3))
        xn_pool = ctx.enter_context(tc.tile_pool(name="xn", bufs=3))
        xt_pool = ctx.enter_context(tc.tile_pool(name="xt", bufs=2))
        ps_pool = ctx.enter_context(
            tc.tile_pool(name="psum", bufs=4, space="PSUM"))
        scr_pool = ctx.enter_context(tc.tile_pool(name="scr", bufs=3))
        fin_pool = ctx.enter_context(tc.tile_pool(name="fin", bufs=1))
        ps1_pool = ctx.enter_context(
            tc.tile_pool(name="psum1", bufs=1, space="PSUM"))

        Y = fin_pool.tile([P, NSAMP, NYC], dt.float32)
        ones = fin_pool.tile([P, 1], dt.float32)
        nc.vector.memset(ones, 1.0)

        # DRAM view for one-shot load: iterate (p, cb, s)
        x_v = x_dram.ap().rearrange("(n cb p) s -> n p cb s", n=NSAMP, cb=CB)

        for n in range(NSAMP):
            # ---- load (cast fp32 -> bf16), one DMA per sample ----
            xbf = xbf_pool.tile([P, CB, S], dt.bfloat16)
            nc.gpsimd.dma_start(out=xbf, in_=x_v[n])

            # ---- per-channel stats ----
            st6 = st_pool.tile([P, CB, 2, 6], dt.float32)
            mv = st_pool.tile([P, CB, 2], dt.float32)
            nrm = st_pool.tile([P, CB], dt.float32)
            sc = st_pool.tile([P, CB], dt.float32)
            for cb in range(CB):
                nc.vector.bn_stats(out=st6[:, cb, 0, :], in_=xbf[:, cb, 0:512])
                nc.vector.bn_stats(out=st6[:, cb, 1, :], in_=xbf[:, cb, 512:S])
                nc.vector.bn_aggr(out=mv[:, cb, :], in_=st6[:, cb, :, :])
                # nrm = sqrt(S * var)
                nc.scalar.activation(out=nrm[:, cb:cb + 1],
                                     in_=mv[:, cb, 1:2],
                                     func=mybir.ActivationFunctionType.Sqrt,
                                     scale=float(S))
                nc.vector.tensor_scalar_add(out=nrm[:, cb:cb + 1],
                                            in0=nrm[:, cb:cb + 1],
                                            scalar1=1e-8)
                nc.vector.reciprocal(out=sc[:, cb:cb + 1],
                                     in_=nrm[:, cb:cb + 1])

            # ---- normalize: xn = (x - mean) * sc  (bf16 out) ----
            xn = xn_pool.tile([P, CB, S], dt.bfloat16)
            for cb in range(CB):
                nc.vector.tensor_scalar(out=xn[:, cb, :], in0=xbf[:, cb, :],
                                        scalar1=mv[:, cb, 0:1],
                                        scalar2=sc[:, cb:cb + 1],
                                        op0=mybir.AluOpType.subtract,
                                        op1=mybir.AluOpType.mult)

            # ---- transpose: ONE whole-sample xbar call, contiguous out ----
            # xn 2D view [128, 8192]: f = cb*S + s. Transposed row f lands at
            # partition f%128, mid f//128 (mid-outer), so the contiguous out
            # tile xT[q, cb, sb, pc] = xn^T(s=sb*128+q, c=cb*128+pc).
            xT = xt_pool.tile([P, CB, KB, P], dt.bfloat16)
            nc.sync.dma_start(
                out=xT.rearrange("q cb sb pc -> q (cb sb) pc"),
                in_=xn.rearrange("p cb s -> p (cb s)"),
                transpose=True)

            # ---- Gram (upper-tri chunks) + Abs-accumulate ----
            # k-block sb: lhsT = xT[:, m, sb, :]  [128, 128]
            #             rhs  = xT[:, cb0:cb1, sb, :]  [128, ncb, 128]
            off_col = N_DIAG_COLS
            for m in range(CB):
                d0 = m * P
                cs = d0
                ci = 0
                while cs < C:
                    w = min(CHUNK, C - cs)
                    cb0, ncb = cs // P, w // P
                    ps = ps_pool.tile([P, CHUNK], dt.float32)
                    for sb in range(KB):
                        nc.tensor.matmul(ps[:, :w],
                                         xT[:, m, sb, :],
                                         xT[:, cb0:cb0 + ncb, sb, :],
                                         start=(sb == 0), stop=(sb == KB - 1))
                    scr = scr_pool.tile([P, CHUNK], dt.float32)
                    if ci == 0:
                        # chunk starts with the diagonal block
                        nc.scalar.activation(
                            out=scr[:, 0:P], in_=ps[:, 0:P],
                            func=mybir.ActivationFunctionType.Abs,
                            accum_out=Y[:, n, m:m + 1])
                        if w > P:
                            nc.scalar.activation(
                                out=scr[:, P:w], in_=ps[:, P:w],
                                func=mybir.ActivationFunctionType.Abs,
                                accum_out=Y[:, n, off_col:off_col + 1])
                            off_col += 1
                    else:
                        nc.scalar.activation(
                            out=scr[:, 0:w], in_=ps[:, 0:w],
                            func=mybir.ActivationFunctionType.Abs,
                            accum_out=Y[:, n, off_col:off_col + 1])
                        off_col += 1
                    cs += w
                    ci += 1
            assert off_col == NYC

        # ---- final reduction: total = sum_p (yd + 2*yo) ----
        yd = fin_pool.tile([P, 1], dt.float32)
        yo = fin_pool.tile([P, 1], dt.float32)
        r = fin_pool.tile([P, 1], dt.float32)
        nc.vector.reduce_sum(out=yd, in_=Y[:, :, 0:N_DIAG_COLS],
                             axis=mybir.AxisListType.XY)
        nc.vector.reduce_sum(out=yo, in_=Y[:, :, N_DIAG_COLS:NYC],
                             axis=mybir.AxisListType.XY)
        nc.vector.tensor_scalar(out=r, in0=yo, scalar1=2.0, scalar2=None,
                                op0=mybir.AluOpType.mult)
        nc.vector.tensor_add(out=r, in0=r, in1=yd)

        ps1 = ps1_pool.tile([1, 1], dt.float32)
        nc.tensor.matmul(ps1, r, ones, start=True, stop=True)
        res_sb = fin_pool.tile([1, 1], dt.float32)
        nc.vector.tensor_copy(out=res_sb, in_=ps1)
        nc.sync.dma_start(out=out_dram[:, :], in_=res_sb)

    nc.compile()
    return nc


def _get_program():
    if "nc" not in _cache:
        _cache["nc"] = _build_program()
    return _cache["nc"]


def kernel(**inputs) -> np.ndarray:
    from concourse.bass_utils import run_bass_kernel_spmd

    x = np.asarray(inputs["x"], dtype=np.float32).reshape(32, C, S)

    nc = _get_program()
    in_maps = [
        {"x": np.ascontiguousarray(
            x[i * NSAMP:(i + 1) * NSAMP].reshape(NSAMP * C, S))}
        for i in range(N_CORES)
    ]
    res = run_bass_kernel_spmd(nc, in_maps, core_ids=list(range(N_CORES)))
    total = sum(float(res.results[i]["out"][0, 0]) for i in range(N_CORES))
    total -= 32.0 * C  # remove diagonal (corr_cc ~= 1.0 each)
    num_combinations = C * (C - 1) // 2
    avg = total / num_combinations / 2.0 / 32.0
    return np.array(avg, dtype=np.float32)
